# revision 11
# baseline (speedup 1.0000x reference)
"""TRN2 Bass kernel for the NonLocal (full N^2 attention) block.

Contract: kernel(**inputs) takes the FULL inputs (x:[4,128,64,64] plus 4x
(W:[128,128], b:[128])) and returns the full [4,128,64,64] output.

Sharding: 8 cores = 4 batches x 2 query-halves (2048 query rows each).
Each core receives the full x[b] (keys/values span all 4096 positions) and
its query slice; outputs are disjoint [128,2048] slices -> no collectives.

Per-core pipeline (fp32 end to end; matmuls read operands as float32r):
  phi   = Wph @ X + bph            [C, 4096]
  theta = Wth @ Xq + bth           [C, 2048]
  gT    = X^T @ Wg^T               [4096, C]   (bias bg folded into bo_eff)
  per 512-wide q-group, streaming over 32 key-chunks of 128:
    scT  = phi_chunk^T @ theta_q   [128, 512]  (scores transposed)
    E    = exp(scT)                            (no max subtraction; scores
                                                are ~N(0,11.3), max ~68, and
                                                exp sums < 3e29 << fp32 max)
    yT  += gT_chunk^T @ E          [C, 512]    (PSUM accumulation)
    sums_acc += E                              (DVE running sum)
  sums  = ones^T @ sums_acc        [1, 512]    (partition reduction on PE)
  y     = yT * (1/sums)                        (lazy softmax normalization)
  out   = sigmoid(Wo @ y + (Wo@bg + bo)) * Xq

All per-core inputs are packed into ONE [128, 6659] DRAM tensor so a single
dma_start (split across all 16 SDMA engines) loads everything: fp32r matmuls
lower to LDW+MM pairs with very few sync-wait slots, so the inputs must
arrive behind a single semaphore.
"""

import sys

for _p in ("/opt/trn_rl_repo",):
    if _p not in sys.path:
        sys.path.insert(0, _p)

import numpy as np

import concourse.bass as bass
import concourse.bacc as bacc
import concourse.mybir as mybir
import concourse.tile as tile
from concourse.bass_utils import run_bass_kernel_spmd

F32 = mybir.dt.float32
F32R = mybir.dt.float32r
AF = mybir.ActivationFunctionType

B, C, H, W = 4, 128, 64, 64
N = H * W            # 4096 key/value positions
NQ = N // 2          # 2048 query rows per core
QG = 512             # q-group width (one PSUM bank / max fp32 moving dim)
NQG = NQ // QG       # 4 q-groups
MC = 32              # key chunks of 128
N_CORES = 8

# packed input column offsets
OFF_XF = 0
OFF_XQ = OFF_XF + N
OFF_WG = OFF_XQ + NQ
OFF_WTH = OFF_WG + C
OFF_WPH = OFF_WTH + C
OFF_WO = OFF_WPH + C
OFF_BTH = OFF_WO + C
OFF_BPH = OFF_BTH + 1
OFF_BO = OFF_BPH + 1
OFF_ONES = OFF_BO + 1
NW = OFF_ONES + C    # 6787 (includes a [C,128] all-ones block)

_CACHE = {}


def build_program():
    nc = bacc.Bacc("TRN2", target_bir_lowering=False, debug=False,
                   num_devices=N_CORES)

    inp = nc.declare_dram_parameter("inp", [C, NW], F32R, isOutput=False)
    out = nc.declare_dram_parameter("out", [C, NQ], F32, isOutput=True)

    with tile.TileContext(nc) as tc:
        with (
            tc.tile_pool(name="const", bufs=1) as const,
            tc.tile_pool(name="big", bufs=1) as big,
            tc.tile_pool(name="epool", bufs=4) as epool,
            tc.tile_pool(name="spool", bufs=2) as spool,
            tc.tile_pool(name="opool", bufs=2) as opool,
            tc.tile_pool(name="ps_sc", bufs=2, space="PSUM") as ps_sc,
            tc.tile_pool(name="ps_y", bufs=2, space="PSUM") as ps_y,
            tc.tile_pool(name="ps_mm", bufs=2, space="PSUM") as ps_mm,
            tc.tile_pool(name="ps_sum", bufs=2, space="PSUM") as ps_sum,
        ):
            inp_s = big.tile([C, NW], F32R)
            nc.sync.dma_start(out=inp_s, in_=inp[:, :])

            xf_s = inp_s[:, OFF_XF:OFF_XF + N]
            xq_s = inp_s[:, OFF_XQ:OFF_XQ + NQ]
            wg_s = inp_s[:, OFF_WG:OFF_WG + C]
            wth_s = inp_s[:, OFF_WTH:OFF_WTH + C]
            wph_s = inp_s[:, OFF_WPH:OFF_WPH + C]
            wo_s = inp_s[:, OFF_WO:OFF_WO + C]
            bth_s = inp_s[:, OFF_BTH:OFF_BTH + 1].bitcast(F32)
            bph_s = inp_s[:, OFF_BPH:OFF_BPH + 1].bitcast(F32)
            bo_s = inp_s[:, OFF_BO:OFF_BO + 1].bitcast(F32)

            ones_s = inp_s[:, OFF_ONES:OFF_ONES + 1]
            ones_row = inp_s[0:1, OFF_ONES:OFF_ONES + C]

            # phi = Wph @ X + bph  [C, N]
            phi_s = big.tile([C, N], F32R)
            for j in range(N // QG):
                ps = ps_mm.tile([C, QG], F32, tag="mm")
                nc.tensor.matmul(
                    ps, lhsT=wph_s, rhs=xf_s[:, j * QG:(j + 1) * QG],
                    start=True, stop=True,
                )
                nc.scalar.activation(
                    out=phi_s[:, j * QG:(j + 1) * QG], in_=ps,
                    func=AF.Identity, bias=bph_s, scale=1.0,
                )

            # theta = Wth @ Xq + bth  [C, NQ]
            th_s = big.tile([C, NQ], F32R)
            for j in range(NQ // QG):
                ps = ps_mm.tile([C, QG], F32, tag="mm")
                nc.tensor.matmul(
                    ps, lhsT=wth_s, rhs=xq_s[:, j * QG:(j + 1) * QG],
                    start=True, stop=True,
                )
                nc.scalar.activation(
                    out=th_s[:, j * QG:(j + 1) * QG], in_=ps,
                    func=AF.Identity, bias=bth_s, scale=1.0,
                )

            # gT[m, c] = sum_ci X[ci, m] * Wg[c, ci]  (no bias; folded into bo_eff)
            gT_s = big.tile([C, MC, C], F32R)
            for mc in range(MC):
                ps = ps_mm.tile([C, QG], F32, tag="mm")
                nc.tensor.matmul(
                    ps[:, :C], lhsT=xf_s[:, mc * C:(mc + 1) * C], rhs=wg_s,
                    start=True, stop=True,
                )
                nc.scalar.copy(out=gT_s[:, mc, :], in_=ps[:, :C])

            # attention, one 512-wide q-group at a time
            for qg in range(NQG):
                q0 = qg * QG
                yt_ps = ps_y.tile([C, QG], F32)
                acc = spool.tile([C, QG], F32R)
                for mc in range(MC):
                    sc = ps_sc.tile([C, QG], F32)
                    nc.tensor.matmul(
                        sc, lhsT=phi_s[:, mc * C:(mc + 1) * C],
                        rhs=th_s[:, q0:q0 + QG], start=True, stop=True,
                    )
                    et = epool.tile([C, QG], F32R)
                    nc.scalar.activation(out=et, in_=sc, func=AF.Exp)
                    nc.tensor.matmul(
                        yt_ps, lhsT=gT_s[:, mc, :], rhs=et,
                        start=(mc == 0), stop=(mc == MC - 1),
                    )
                    if mc == 0:
                        nc.vector.tensor_copy(out=acc, in_=et)
                    else:
                        nc.vector.tensor_add(out=acc, in0=acc, in1=et)

                sums_ps = ps_sum.tile([1, QG], F32)
                nc.tensor.matmul(
                    sums_ps, lhsT=ones_s, rhs=acc, start=True, stop=True,
                )
                recip_s = spool.tile([1, QG], F32R)
                with nc.allow_low_precision(reason="f32r rounding of 1/sums"):
                    nc.vector.reciprocal(out=recip_s, in_=sums_ps)
                # broadcast 1/sums across partitions via a K=1 matmul
                rb_ps = ps_mm.tile([C, QG], F32, tag="mm")
                nc.tensor.matmul(
                    rb_ps, lhsT=ones_row, rhs=recip_s, start=True, stop=True,
                )
                rb_s = spool.tile([C, QG], F32)
                nc.scalar.copy(out=rb_s, in_=rb_ps)
                yn_s = opool.tile([C, QG], F32R)
                with nc.allow_low_precision(reason="f32r rounding of y"):
                    nc.vector.tensor_mul(out=yn_s, in0=yt_ps, in1=rb_s)

                wy_ps = ps_mm.tile([C, QG], F32, tag="mm")
                nc.tensor.matmul(
                    wy_ps, lhsT=wo_s, rhs=yn_s, start=True, stop=True,
                )
                sig_s = opool.tile([C, QG], F32)
                nc.scalar.activation(
                    out=sig_s, in_=wy_ps, func=AF.Sigmoid, bias=bo_s, scale=1.0,
                )
                o_s = opool.tile([C, QG], F32)
                nc.vector.tensor_mul(out=o_s, in0=sig_s, in1=xq_s[:, q0:q0 + QG].bitcast(F32))
                nc.sync.dma_start(out=out[:, q0:q0 + QG], in_=o_s)

    nc.compile()
    return nc


def get_program():
    if "nc" not in _CACHE:
        _CACHE["nc"] = build_program()
    return _CACHE["nc"]


def make_in_maps(x, Wg, bg, Wth, bth, Wph, bph, Wo, bo):
    xr = np.ascontiguousarray(x.reshape(B, C, N), np.float32)
    bo_eff = (Wo.astype(np.float64) @ bg.astype(np.float64)
              + bo.astype(np.float64)).astype(np.float32)
    wblock = np.concatenate([
        np.ascontiguousarray(Wg.T, np.float32),
        np.ascontiguousarray(Wth.T, np.float32),
        np.ascontiguousarray(Wph.T, np.float32),
        np.ascontiguousarray(Wo.T, np.float32),
        bth.reshape(C, 1).astype(np.float32),
        bph.reshape(C, 1).astype(np.float32),
        bo_eff.reshape(C, 1),
        np.ones((C, C), np.float32),
    ], axis=1)
    in_maps = []
    for core in range(N_CORES):
        b, qh = divmod(core, 2)
        packed = np.concatenate(
            [xr[b], xr[b][:, qh * NQ:(qh + 1) * NQ], wblock], axis=1)
        in_maps.append({"inp": np.ascontiguousarray(packed)})
    return in_maps


def run(trace=False, **inputs):
    nc = get_program()
    in_maps = make_in_maps(**inputs)
    res = run_bass_kernel_spmd(nc, in_maps, core_ids=list(range(N_CORES)),
                               trace=trace)
    full = np.empty((B, C, N), np.float32)
    for core in range(N_CORES):
        b, qh = divmod(core, 2)
        full[b][:, qh * NQ:(qh + 1) * NQ] = res.results[core]["out"]
    return full.reshape(B, C, H, W), res


def kernel(**inputs) -> np.ndarray:
    out, _ = run(trace=False, **inputs)
    return out


# revision 12
# speedup vs baseline: 1.2902x; 1.2902x over previous
"""TRN2 Bass kernel for the NonLocal (full N^2 attention) block.

Contract: kernel(**inputs) takes the FULL inputs (x:[4,128,64,64] plus 4x
(W:[128,128], b:[128])) and returns the full [4,128,64,64] output.

Sharding: 8 cores = 4 batches x 2 query-halves (2048 query rows each).
Each core receives the full x[b] (keys/values span all 4096 positions) and
its query slice; outputs are disjoint [128,2048] slices -> no collectives.

Per-core pipeline:
  phi   = Wph @ X + bph            [C, 4096]   (f32r conv, SC_DT output)
  theta = Wth @ Xq + bth           [C, 2048]
  gT    = X^T @ Wg^T               [4096, C]   (bias bg folded into bo_eff)
  per 1024-wide q-group, streaming over 32 key-chunks of 128:
    scT  = phi_chunk^T @ theta_q   [128, 1024] (scores transposed, 2 matmuls)
    E    = exp(scT)                one ACT op  (no max subtraction; scores
                                                are ~N(0,11.3), max ~68, and
                                                exp sums < 3e29 << fp32 max)
    yT  += gT_chunk^T @ E          [C, 1024]   (PSUM accumulation, 2 matmuls)
    sums_acc += E                               (DVE running sum / bf16 tree)
  sums  = allones^T @ sums_acc     [C, 1024]   (partition reduce + broadcast
                                                in ONE matmul: every output
                                                partition gets the col sum)
  y     = yT * recip_approx(sums)              (lazy softmax normalization)
  out   = sigmoid(Wo @ y + (Wo@bg + bo)) * Xq

The q-group tail is emitted a few chunks INTO the next q-group so the PE
stream never stalls on the DVE tail (stalling >3.4us re-throttles the PE
clock to 1.2GHz via HAM).

All per-core inputs are packed into ONE [128, 6787] DRAM tensor so a single
dma_start (split across all 16 SDMA engines) loads everything behind one
semaphore (fp32r matmuls have very few sync-wait slots).
"""

import sys

for _p in ("/opt/trn_rl_repo",):
    if _p not in sys.path:
        sys.path.insert(0, _p)

import numpy as np

import concourse.bass as bass
import concourse.bacc as bacc
import concourse.mybir as mybir
import concourse.tile as tile
from concourse.bass_utils import run_bass_kernel_spmd

F32 = mybir.dt.float32
F32R = mybir.dt.float32r
F16 = mybir.dt.float16
BF16 = mybir.dt.bfloat16
AF = mybir.ActivationFunctionType

B, C, H, W = 4, 128, 64, 64
N = H * W            # 4096 key/value positions
NQ = N // 2          # 2048 query rows per core
QG = 512             # PSUM bank / max fp32 moving dim
GW = 2 * QG          # q-group width (1024)
NQG = NQ // GW       # 2 q-groups
MC = 32              # key chunks of 128
N_CORES = 8

# dtype knobs:
#  SC_DT: theta/phi storage + scores matmul operands. F32R (~2cyc/row, m11)
#         or F16 (1cyc/row, m10).
#  AV_DT: exp output + gT storage + AV matmul operands. F32R or BF16.
#  SUMS_TREE: False -> fp32 running accumulator (DVE 1x adds);
#             True  -> bf16 pairwise tree (DVE 2x adds; needs AV_DT=BF16).
SC_DT = F16
AV_DT = BF16
SUMS_TREE = True

# packed input column offsets
OFF_XF = 0
OFF_XQ = OFF_XF + N
OFF_WG = OFF_XQ + NQ
OFF_WTH = OFF_WG + C
OFF_WPH = OFF_WTH + C
OFF_WO = OFF_WPH + C
OFF_BTH = OFF_WO + C
OFF_BPH = OFF_BTH + 1
OFF_BO = OFF_BPH + 1
OFF_ONES = OFF_BO + 1
NW = OFF_ONES + C    # 6787 (includes a [C,128] all-ones block)

_CACHE = {}


def build_program():
    nc = bacc.Bacc("TRN2", target_bir_lowering=False, debug=False,
                   num_devices=N_CORES)

    inp = nc.declare_dram_parameter("inp", [C, NW], F32R, isOutput=False)
    out = nc.declare_dram_parameter("out", [C, NQ], F32, isOutput=True)

    with tile.TileContext(nc) as tc:
        with (
            tc.tile_pool(name="const", bufs=1) as const,
            tc.tile_pool(name="big", bufs=1) as big,
            tc.tile_pool(name="epool", bufs=4) as epool,
            tc.tile_pool(name="tpool", bufs=10) as tpool,
            tc.tile_pool(name="tailp", bufs=2) as tailp,
        ):
            inp_s = big.tile([C, NW], F32R)
            # weights/biases/xq first, bulk xf second: theta conv can start
            # while xf is still streaming in
            nc.sync.dma_start(out=inp_s[:, OFF_XQ:], in_=inp[:, OFF_XQ:])
            nc.sync.dma_start(out=inp_s[:, :OFF_XQ], in_=inp[:, :OFF_XQ])

            xf_s = inp_s[:, OFF_XF:OFF_XF + N]
            xq_s = inp_s[:, OFF_XQ:OFF_XQ + NQ]
            wg_s = inp_s[:, OFF_WG:OFF_WG + C]
            wth_s = inp_s[:, OFF_WTH:OFF_WTH + C]
            wph_s = inp_s[:, OFF_WPH:OFF_WPH + C]
            wo_s = inp_s[:, OFF_WO:OFF_WO + C]
            bth_s = inp_s[:, OFF_BTH:OFF_BTH + 1].bitcast(F32)
            bph_s = inp_s[:, OFF_BPH:OFF_BPH + 1].bitcast(F32)
            bo_s = inp_s[:, OFF_BO:OFF_BO + 1].bitcast(F32)
            ones_blk = inp_s[:, OFF_ONES:OFF_ONES + C]

            if SUMS_TREE:
                ones_sum = const.tile([C, C], AV_DT)
                nc.vector.memset(ones_sum, 1.0)
            else:
                ones_sum = ones_blk

            th_s = big.tile([C, NQ], SC_DT)
            phi_s = big.tile([C, N], SC_DT)
            gT_s = big.tile([C, MC, C], AV_DT)

            # ---- convs (own PSUM pool scope so banks free up afterwards) ----
            with tc.tile_pool(name="ps_conv", bufs=3, space="PSUM") as ps_conv:
                # theta = Wth @ Xq + bth (only needs xq + weights)
                for j in range(NQ // QG):
                    ps = ps_conv.tile([C, QG], F32, tag="cv")
                    nc.tensor.matmul(
                        ps, lhsT=wth_s, rhs=xq_s[:, j * QG:(j + 1) * QG],
                        start=True, stop=True,
                    )
                    with nc.allow_low_precision(reason="theta storage dtype"):
                        nc.scalar.activation(
                            out=th_s[:, j * QG:(j + 1) * QG], in_=ps,
                            func=AF.Identity, bias=bth_s, scale=1.0,
                        )
                # phi = Wph @ X + bph
                for j in range(N // QG):
                    ps = ps_conv.tile([C, QG], F32, tag="cv")
                    nc.tensor.matmul(
                        ps, lhsT=wph_s, rhs=xf_s[:, j * QG:(j + 1) * QG],
                        start=True, stop=True,
                    )
                    with nc.allow_low_precision(reason="phi storage dtype"):
                        nc.scalar.activation(
                            out=phi_s[:, j * QG:(j + 1) * QG], in_=ps,
                            func=AF.Identity, bias=bph_s, scale=1.0,
                        )
                # gT[m, c] (bias folded into bo_eff on the host)
                for mc in range(MC):
                    ps = ps_conv.tile([C, QG], F32, tag="cv")
                    nc.tensor.matmul(
                        ps[:, :C], lhsT=xf_s[:, mc * C:(mc + 1) * C], rhs=wg_s,
                        start=True, stop=True,
                    )
                    with nc.allow_low_precision(reason="gT storage dtype"):
                        nc.vector.tensor_copy(out=gT_s[:, mc, :], in_=ps[:, :C])

            # ---- attention ----
            with (
                tc.tile_pool(name="ps_sc", bufs=2, space="PSUM") as ps_sc,
                tc.tile_pool(name="ps_y", bufs=1, space="PSUM") as ps_y,
                tc.tile_pool(name="ps_mm", bufs=2, space="PSUM") as ps_mm,
            ):
                pending_tail = []

                def emit_tail(qg, yt_ps, sums_tile):
                    q0 = qg * GW
                    for h in range(2):
                        sl = slice(h * QG, (h + 1) * QG)
                        rb = ps_mm.tile([C, QG], F32, tag="mm", name=f"rb_{qg}_{h}")
                        nc.tensor.matmul(
                            rb, lhsT=ones_sum, rhs=sums_tile[:, sl],
                            start=True, stop=True,
                        )
                        rbi = tailp.tile([C, QG], F32, name=f"rbi_{qg}_{h}", tag="rbi")
                        nc.vector.reciprocal_approx_fast(out=rbi, in_=rb)
                        yn = tailp.tile([C, QG], F32R, name=f"yn_{qg}_{h}", tag="yn")
                        with nc.allow_low_precision(reason="f32r rounding of y"):
                            nc.vector.tensor_mul(out=yn, in0=yt_ps[:, sl], in1=rbi)
                        wy = ps_mm.tile([C, QG], F32, tag="mm", name=f"wy_{qg}_{h}")
                        nc.tensor.matmul(
                            wy, lhsT=wo_s, rhs=yn, start=True, stop=True,
                        )
                        sig = tailp.tile([C, QG], F32, name=f"sig_{qg}_{h}", tag="sig")
                        nc.scalar.activation(
                            out=sig, in_=wy, func=AF.Sigmoid, bias=bo_s, scale=1.0,
                        )
                        o = tailp.tile([C, QG], F32, name=f"o_{qg}_{h}", tag="o")
                        nc.vector.tensor_mul(
                            out=o, in0=sig,
                            in1=xq_s[:, q0 + h * QG:q0 + (h + 1) * QG].bitcast(F32),
                        )
                        nc.sync.dma_start(
                            out=out[:, q0 + h * QG:q0 + (h + 1) * QG], in_=o)

                for qg in range(NQG):
                    q0 = qg * GW
                    yt_ps = ps_y.tile([C, GW], F32, name=f"yt_{qg}", tag="yt")
                    acc = None           # fp32 running sum (SUMS_TREE=False)
                    levels = [None] * 6  # bf16 binary-counter tree
                    tree_n = 0

                    for mc in range(MC):
                        sc = ps_sc.tile([C, GW], F32, name=f"sc_{qg}_{mc}", tag="sc")
                        for h in range(2):
                            sl = slice(h * QG, (h + 1) * QG)
                            nc.tensor.matmul(
                                sc[:, sl], lhsT=phi_s[:, mc * C:(mc + 1) * C],
                                rhs=th_s[:, q0 + h * QG:q0 + (h + 1) * QG],
                                start=True, stop=True,
                            )
                        et = epool.tile([C, GW], AV_DT, name=f"et_{qg}_{mc}", tag="et")
                        with nc.allow_low_precision(reason="exp output dtype"):
                            nc.scalar.activation(out=et, in_=sc, func=AF.Exp)
                        for h in range(2):
                            sl = slice(h * QG, (h + 1) * QG)
                            nc.tensor.matmul(
                                yt_ps[:, sl], lhsT=gT_s[:, mc, :], rhs=et[:, sl],
                                start=(mc == 0), stop=(mc == MC - 1),
                            )
                        if SUMS_TREE:
                            cur = et
                            lvl = 0
                            while levels[lvl] is not None:
                                t = tpool.tile([C, GW], AV_DT,
                                               name=f"tree_{qg}_{mc}_{lvl}",
                                               tag="tree")
                                with nc.allow_low_precision(reason="bf16 sum tree"):
                                    nc.vector.tensor_add(
                                        out=t, in0=levels[lvl], in1=cur)
                                levels[lvl] = None
                                cur = t
                                lvl += 1
                            levels[lvl] = cur
                            tree_n += 1
                        else:
                            if acc is None:
                                acc = tpool.tile([C, GW], F32R,
                                                 name=f"acc_{qg}", tag="acc")
                                with nc.allow_low_precision(reason="f32r sums acc"):
                                    nc.vector.tensor_copy(out=acc, in_=et)
                            else:
                                with nc.allow_low_precision(reason="f32r sums acc"):
                                    nc.vector.tensor_add(out=acc, in0=acc, in1=et)

                        if mc == 3 and pending_tail:
                            emit_tail(*pending_tail.pop())

                    if SUMS_TREE:
                        live = [t for t in levels if t is not None]
                        cur = live[0]
                        for i, t in enumerate(live[1:]):
                            nt = tpool.tile([C, GW], AV_DT,
                                            name=f"treeflush_{qg}_{i}", tag="tree")
                            with nc.allow_low_precision(reason="bf16 sum tree"):
                                nc.vector.tensor_add(out=nt, in0=cur, in1=t)
                            cur = nt
                        sums_tile = cur
                    else:
                        sums_tile = acc

                    pending_tail.append((qg, yt_ps, sums_tile))

                emit_tail(*pending_tail.pop())

    nc.compile()
    return nc


def get_program():
    if "nc" not in _CACHE:
        _CACHE["nc"] = build_program()
    return _CACHE["nc"]


def make_in_maps(x, Wg, bg, Wth, bth, Wph, bph, Wo, bo):
    xr = np.ascontiguousarray(x.reshape(B, C, N), np.float32)
    bo_eff = (Wo.astype(np.float64) @ bg.astype(np.float64)
              + bo.astype(np.float64)).astype(np.float32)
    wblock = np.concatenate([
        np.ascontiguousarray(Wg.T, np.float32),
        np.ascontiguousarray(Wth.T, np.float32),
        np.ascontiguousarray(Wph.T, np.float32),
        np.ascontiguousarray(Wo.T, np.float32),
        bth.reshape(C, 1).astype(np.float32),
        bph.reshape(C, 1).astype(np.float32),
        bo_eff.reshape(C, 1),
        np.ones((C, C), np.float32),
    ], axis=1)
    in_maps = []
    for core in range(N_CORES):
        b, qh = divmod(core, 2)
        packed = np.concatenate(
            [xr[b], xr[b][:, qh * NQ:(qh + 1) * NQ], wblock], axis=1)
        in_maps.append({"inp": np.ascontiguousarray(packed)})
    return in_maps


def run(trace=False, **inputs):
    nc = get_program()
    in_maps = make_in_maps(**inputs)
    res = run_bass_kernel_spmd(nc, in_maps, core_ids=list(range(N_CORES)),
                               trace=trace)
    full = np.empty((B, C, N), np.float32)
    for core in range(N_CORES):
        b, qh = divmod(core, 2)
        full[b][:, qh * NQ:(qh + 1) * NQ] = res.results[core]["out"]
    return full.reshape(B, C, H, W), res


def kernel(**inputs) -> np.ndarray:
    out, _ = run(trace=False, **inputs)
    return out
